# revision 48
# baseline (speedup 1.0000x reference)
"""GAT 2-layer model on 8 Trainium2 NeuronCores (Bass/Tile) — v3.

Strategy (vs v2): nodes dst-sharded across 8 cores; edges of core p grouped
by (dst-window of 128, src-block of 25k), cell-packed into C chunks of 128
slots. fc1 replicated: each core computes the full fc1+att table in its HBM
(t1, 512B rows: 16 f32 el+er | 256 fp8 feat | pad — fp8 feats halve the
dominant gather traffic, validated ~5e-3 end-to-end err vs 2e-2 budget).
Per window-pair, dma_gathers are MERGED across GRP windows (amortizing the
~1us fixed SWDGE cost per gather; single_packet only when <=64 descs per
engine — more crashes the DGE). er[dst] read back via per-window [128,1]
indirect gathers (multi-column idx orders descriptors differently on HW
than the interp — do not batch). One-hot st built broadcast on DVE; stT
built FLAT vs an iotapx constant (1.75x faster than broadcast APs). st@msg
segment-sum in PSUM; leaky-relu+exp as max(exp(v), exp(0.2v)) so ACT only
runs Exp/Copy; post copies on ACT to offload the DVE bottleneck. conv2
repeats on the AllGathered t2 table (bf16, 256B rows).
"""
import sys

for _p in ("/opt/trn_rl_repo",):
    if _p not in sys.path:
        sys.path.insert(0, _p)

import math
from dataclasses import dataclass

import numpy as np
import ml_dtypes

import concourse.bass as bass
import concourse.bacc as bacc
import concourse.mybir as mybir
import concourse.tile as tile
from concourse.bass_utils import run_bass_kernel_spmd

BF16 = ml_dtypes.bfloat16
FP8 = ml_dtypes.float8_e4m3
NCORES = 8
NQ = 4  # src blocks
NEG = 0.2

IN_F = 128
HID = 32
HEADS = 8
OUT_F = 64
F1 = HEADS * HID          # 256
T1W = 512                 # fp8 bytes per T1 row: 256 feat fp8 | 32B (el f32) | pad
T2W = 128                 # bf16 cols per T2 row: 64 feat | 2 (el2 f32) | pad
SCRATCH = 32768           # dynamic DMA scratch (descriptor ring carveout)
NQUEUES = 4
GRP1 = 2                  # conv1 windows per merged gather group
GRP2 = 2                  # conv2 windows per merged gather group


@dataclass(frozen=True)
class Cfg:
    n: int
    e: int
    c: int  # slot chunks (x128) per (window, src-block) cell

    @property
    def ln(self):
        return self.n // NCORES

    @property
    def nw(self):
        return math.ceil(self.ln / 128)

    @property
    def t2ln(self):
        return self.nw * 128

    @property
    def blkn(self):
        return self.n // NQ

    @property
    def t2blk(self):
        return 2 * self.t2ln

    @property
    def npad(self):
        return 1024 * math.ceil(self.n / 1024)

    @property
    def nbatch(self):
        return self.npad // 1024

    @property
    def ic(self):  # int16 idx cols per (window, q) cell
        return self.c * 128 // 16

    @property
    def nch(self):
        return NQ * self.c

    @property
    def meta_w(self):  # int16 cols: dmod[NCH] | xblk[NCH*128]
        w = self.nch + self.nch * 128
        return 16 * math.ceil(w / 16)


def _fold(al, heads, hid):
    a = np.zeros((heads * hid, heads), np.float32)
    for h in range(heads):
        a[h * hid:(h + 1) * hid, h] = al[h]
    return a


def _wrap_idx(flat):
    """[n] int -> [128, n//16] int16: wrapped in 16 partitions, replicated 8x."""
    n = len(flat)
    w = np.asarray(flat, np.int16).reshape(n // 16, 16).T
    return np.tile(w, (8, 1))


def prep(inputs, cfg: Cfg | None = None):
    """Host-side: fold weights, pack per-core edge schedules."""
    feats = np.asarray(inputs["features"], np.float32)
    src = np.asarray(inputs["src"], np.int64)
    dst = np.asarray(inputs["dst"], np.int64)
    n, e = feats.shape[0], src.shape[0]

    W1f = np.concatenate(
        [inputs["W1"],
         inputs["W1"] @ _fold(np.asarray(inputs["al1"]), HEADS, HID),
         inputs["W1"] @ _fold(np.asarray(inputs["ar1"]), HEADS, HID)], axis=1
    ).astype(np.float32)  # [128, 272]
    W2f = np.concatenate(
        [inputs["W2"],
         inputs["W2"] @ _fold(np.asarray(inputs["al2"]), 1, OUT_F),
         inputs["W2"] @ _fold(np.asarray(inputs["ar2"]), 1, OUT_F)], axis=1
    ).astype(np.float32)  # [256, 66]

    ln = n // NCORES
    core = dst // ln
    nw = math.ceil(ln / 128)
    blkn = n // NQ
    w_of = (dst - core * ln) // 128
    q_of = src // blkn
    cell = ((core * nw + w_of) * NQ + q_of).astype(np.int64)
    counts = np.bincount(cell, minlength=NCORES * nw * NQ)
    c_need = math.ceil(counts.max() / 128)
    if cfg is None:
        cfg = Cfg(n=n, e=e, c=int(c_need))
    assert counts.max() <= cfg.c * 128, (counts.max(), cfg.c)
    C = cfg.c

    t2row_src = (src // ln) * cfg.t2ln + (src % ln)

    order = np.lexsort((q_of, w_of, core))
    src_s, dst_s = src[order], dst[order]
    core_s, w_s, q_s = core[order], w_of[order], q_of[order]
    t2src_s = t2row_src[order]

    in_maps = []
    featT = np.zeros((128, cfg.npad), BF16)
    featT[:, :n] = feats.T.astype(BF16)
    iota_rep = np.tile(np.arange(128, dtype=np.float32), (128, 1)).astype(BF16)
    iota_p = np.arange(128, dtype=np.float32)[:, None].astype(BF16)  # [128,1]
    iota_px = np.tile(np.arange(128, dtype=np.float32)[:, None],
                      (1, NQ * cfg.c * 128)).astype(BF16)  # [128, NCH*128]
    ident = np.eye(128, dtype=np.float32).astype(BF16)
    w1f_b = W1f.astype(BF16)
    w2f_b = np.zeros((128, 2, 66), BF16)
    w2f_b[:, 0, :] = W2f[:128].astype(BF16)
    w2f_b[:, 1, :] = W2f[128:].astype(BF16)
    b1_rep = np.tile(np.asarray(inputs["b1"], np.float32)[None, :], (128, 1))
    b2_rep = np.tile(np.asarray(inputs["b2"], np.float32)[None, :], (128, 1))
    pW_rep = np.tile(np.asarray(inputs["pW"], np.float32)[:, 0][None, :], (128, 1))
    pb_t = np.full((128, 1), float(np.asarray(inputs["pb"])[0]), np.float32)

    ic = cfg.ic
    for p in range(NCORES):
        sel = core_s == p
        es, ed, ew, eq, et2 = src_s[sel], dst_s[sel], w_s[sel], q_s[sel], t2src_s[sel]
        dl = ed - p * ln
        s1 = np.zeros((nw, NQ, C * 128), np.int16)
        s2 = np.zeros((nw, NQ, C * 128), np.int16)
        dm = np.full((nw, NQ, C * 128), 200.0, np.float32)
        ord2 = np.lexsort((eq, ew))
        ew2, eq2 = ew[ord2], eq[ord2]
        cellid = ew2 * NQ + eq2
        pos = np.arange(len(cellid)) - np.concatenate(
            ([0], np.cumsum(np.bincount(cellid, minlength=nw * NQ))))[cellid]
        s1[ew2, eq2, pos] = (es[ord2] - eq2 * blkn).astype(np.int16)
        s2[ew2, eq2, pos] = (et2[ord2] - eq2 * cfg.t2blk).astype(np.int16)
        dm[ew2, eq2, pos] = (dl[ord2] % 128).astype(np.float32)

        idx1 = np.zeros((nw, 128, NQ, ic), np.int16)
        idx2 = np.zeros((nw, 128, NQ, ic), np.int16)
        meta = np.zeros((nw, 128, cfg.meta_w), np.int16)
        for w in range(nw):
            for q in range(NQ):
                idx1[w, :, q, :] = _wrap_idx(s1[w, q])
                idx2[w, :, q, :] = _wrap_idx(s2[w, q])
            dmw = dm[w].reshape(NQ * C, 128)  # [t, s]
            meta[w, :, 0:NQ * C] = dmw.T.astype(BF16).view(np.int16)
            xrow = dmw.reshape(-1).astype(BF16).view(np.int16)
            meta[w, :, NQ * C:NQ * C + NQ * C * 128] = xrow[None, :]

        gid = p * ln + np.arange(cfg.t2ln, dtype=np.int32)
        gid[ln:] = 0
        er_gidx = gid.reshape(nw, 128).T.astype(np.int32)  # [128, nw]
        in_maps.append(dict(
            featT=featT, w1f=w1f_b, w2f=w2f_b, iota=iota_rep, iotap=iota_p,
            iotapx=iota_px,
            ident=ident, b1=b1_rep, b2=b2_rep, pw=pW_rep, pb=pb_t,
            idx1=idx1, idx2=idx2, meta=meta, er_gidx=er_gidx,
        ))
    return cfg, in_maps


def build(cfg: Cfg, stop_after: int = 99, parts: int = 99, repeat: int = 1,
          sim: bool = False, stt_flat: bool = True, lazy_memset: bool = True,
          st2_pool: bool = False, fc1_act_copies: bool = False,
          cv_bufs: int = 3, grp1: int = GRP1, grp2: int = GRP2):
    dt = mybir.dt
    nc = bacc.Bacc("TRN2", target_bir_lowering=False, debug=False,
                   num_devices=NCORES, dynamic_dma_scratch_size=SCRATCH,
                   num_swdge_queues=NQUEUES)
    ap = {}
    def inp(name, shape, dtype):
        ap[name] = nc.dram_tensor(name, shape, dtype, kind="ExternalInput").ap()
    inp("featT", [128, cfg.npad], dt.bfloat16)
    inp("w1f", [128, 272], dt.bfloat16)
    inp("w2f", [128, 2, 66], dt.bfloat16)
    inp("iota", [128, 128], dt.bfloat16)
    inp("iotap", [128, 1], dt.bfloat16)
    inp("iotapx", [128, NQ * cfg.c * 128], dt.bfloat16)
    inp("ident", [128, 128], dt.bfloat16)
    inp("b1", [128, F1], dt.float32)
    inp("b2", [128, OUT_F], dt.float32)
    inp("pw", [128, OUT_F], dt.float32)
    inp("pb", [128, 1], dt.float32)
    inp("idx1", [cfg.nw, 128, NQ, cfg.ic], dt.int16)
    inp("idx2", [cfg.nw, 128, NQ, cfg.ic], dt.int16)
    inp("meta", [cfg.nw, 128, cfg.meta_w], dt.int16)
    inp("er_gidx", [128, cfg.nw], dt.int32)
    out_core = nc.dram_tensor("out_core", [128, cfg.nw], dt.float32,
                              kind="ExternalOutput").ap()

    t1 = nc.dram_tensor("t1", [cfg.npad, T1W], dt.float8e4, kind="Internal").ap()
    t2_loc = nc.dram_tensor("t2l", [cfg.t2ln, T2W], dt.bfloat16, kind="Internal").ap()
    t2_full = nc.dram_tensor("t2f", [NCORES * cfg.t2ln, T2W], dt.bfloat16,
                             kind="Internal", addr_space="Shared").ap()

    C = cfg.c
    ic = cfg.ic
    NCH = NQ * C  # chunks per window
    nblk_rows = [cfg.blkn] * (NQ - 1) + [cfg.npad - (NQ - 1) * cfg.blkn]

    with tile.TileContext(nc) as tc:
        with tc.tile_pool(name="persist", bufs=1) as pp:
            out_acc = pp.tile([128, cfg.nw], dt.float32, tag="out_acc")
            nc.gpsimd.memset(out_acc[:], 0.0)
            w1f_t = pp.tile([128, 272], dt.bfloat16, tag="w1f")
            nc.sync.dma_start(w1f_t[:], ap["w1f"][:])
            w2f_t = pp.tile([128, 2, 66], dt.bfloat16, tag="w2f")
            nc.sync.dma_start(w2f_t[:], ap["w2f"][:])
            iota_t = pp.tile([128, 128], dt.bfloat16, tag="iota")
            nc.sync.dma_start(iota_t[:], ap["iota"][:])
            iotap_t = pp.tile([128, 1], dt.bfloat16, tag="iotap")
            nc.sync.dma_start(iotap_t[:], ap["iotap"][:])
            iotapx_t = pp.tile([128, NQ * cfg.c * 128], dt.bfloat16, tag="iotapx")
            nc.sync.dma_start(iotapx_t[:], ap["iotapx"][:])
            ident_t = pp.tile([128, 128], dt.bfloat16, tag="ident")
            nc.sync.dma_start(ident_t[:], ap["ident"][:])
            b1_t = pp.tile([128, F1], dt.float32, tag="b1")
            nc.sync.dma_start(b1_t[:], ap["b1"][:])
            b2_t = pp.tile([128, OUT_F], dt.float32, tag="b2")
            nc.sync.dma_start(b2_t[:], ap["b2"][:])
            pw_t = pp.tile([128, OUT_F], dt.float32, tag="pw")
            nc.sync.dma_start(pw_t[:], ap["pw"][:])
            pb_t = pp.tile([128, 1], dt.float32, tag="pb")
            nc.sync.dma_start(pb_t[:], ap["pb"][:])
            gidx_t = pp.tile([128, cfg.nw], dt.int32, tag="gidx")
            nc.sync.dma_start(gidx_t[:], ap["er_gidx"][:])
            er1b_t = pp.tile([128, cfg.nw, 8], dt.bfloat16, tag="er1b")
            er2b_t = pp.tile([128, cfg.nw, 1], dt.bfloat16, tag="er2b")

            for _rep in range(repeat):
              # ---------- phase 0: fc1 over all nodes -> t1, er1f ----------
              with (
                  tc.tile_pool(name="p0", bufs=2) as p0,
                  tc.tile_pool(name="p0ps", bufs=4, space="PSUM") as p0ps,
              ):
                  for b in range(cfg.nbatch):
                      fsl = p0.tile([128, 1024], dt.bfloat16, tag="fsl")
                      nc.sync.dma_start(fsl[:], ap["featT"][:, b * 1024:(b + 1) * 1024])
                      stg = p0.tile([128, 8, T1W], dt.float8e4, tag="stg")
                      if not lazy_memset or b < 2:
                          # pool recycles 2 bufs; pad cols never overwritten
                          nc.gpsimd.memset(stg[:, :, 64 + F1:T1W], 0.0)
                      for c in range(8):
                          ps = p0ps.tile([128, 272], dt.float32, space="PSUM", tag="ps")
                          nc.tensor.matmul(out=ps[:], lhsT=fsl[:, c * 128:(c + 1) * 128],
                                           rhs=w1f_t[:], start=True, stop=True)
                          if fc1_act_copies or c % 2 == 0:
                              nc.scalar.activation(stg[:, c, 64:64 + F1],
                                                   ps[:, 0:F1],
                                                   mybir.ActivationFunctionType.Copy)
                          else:
                              nc.vector.tensor_copy(stg[:, c, 64:64 + F1],
                                                    ps[:, 0:F1])
                          nc.vector.tensor_copy(
                              stg[:, c, :].bitcast(dt.float32)[:, 0:16],
                              ps[:, F1:F1 + 16])
                      nc.sync.dma_start(
                          t1[b * 1024:(b + 1) * 1024, :].rearrange(
                              "(c p) e -> p c e", p=128), stg[:])
              tc.strict_bb_all_engine_barrier()

              # ---------- phase 0b: local er1 -> SBUF (bf16) ----------
              # er lives in t1 rows at fp8 cols [288:320] (f32 [72:80]); one
              # batched indirect gather for all nw windows.
              # NOTE: indirect_dma_start ignores the out AP base offset on HW
              # ucode — gather into a fresh tile at offset 0, then copy.
              if stop_after >= 2:
                # NOTE: indirect_dma_start ignores the out AP base offset on
                # HW ucode — gather into a fresh tile at offset 0, then copy.
                # One [128,1]-idx gather per window (multi-column idx orders
                # descriptors differently on HW than the interp).
                with tc.tile_pool(name="erb", bufs=4) as erb:
                  for k in range(cfg.nw):
                      ert = erb.tile([128, 64], dt.float8e4, tag="ert")
                      nc.gpsimd.indirect_dma_start(
                          out=ert[:], out_offset=None, in_=t1[:],
                          in_offset=bass.IndirectOffsetOnAxis(
                              ap=gidx_t[:, k:k + 1], axis=0))
                      nc.vector.tensor_copy(
                          er1b_t[:, k, :],
                          ert[:].bitcast(dt.float32)[:, 8:16])
                tc.strict_bb_all_engine_barrier()

              # ---------- shared conv machinery ----------
              def conv_group(w0, gsz, G, t_src, erb_t, idx_name, fw, agg_cols,
                             post, cv, cvps, first):
                  """Gathers merged across a group of G windows (amortizes the
                  ~1us fixed SWDGE cost per dma_gather), then per-window
                  softmax+aggregate. fw: feature cols in gathered row (fp8 for
                  conv1, bf16 for conv2); el f32 follows the feat block."""
                  nh = agg_cols - fw
                  conv1 = fw == F1
                  elem = T1W if conv1 else T2W
                  gdt = dt.float8e4 if conv1 else dt.bfloat16
                  el_f32_off = 0 if conv1 else fw // 2
                  feat_off = 64 if conv1 else 0
                  ggt = cv.tile([128, NQ, G, C, elem], gdt, tag=f"ggt{elem}")
                  if first:
                      nc.gpsimd.memset(ggt[:], 0.0)
                  idxt = cv.tile([128, NQ, G, ic], dt.int16, tag=f"idxt{elem}")
                  for q in range(NQ):
                      nc.sync.dma_start(
                          idxt[:, q, :gsz, :],
                          ap[idx_name][w0:w0 + gsz, :, q, :].rearrange(
                              "g p i -> p g i"))
                  for q in range(NQ):
                      nc.gpsimd.dma_gather(
                          ggt[:, q, :gsz, :, :].rearrange("p g c e -> p (g c) e"),
                          t_src[q],
                          idxt[:, q, :gsz, :].rearrange("p g i -> p (g i)"),
                          gsz * C * 128, gsz * C * 128, elem,
                          # >64 descs per engine breaks single_packet mode
                          single_packet=(gsz * C * 128 // 16 + 1 <= 64),
                          queue_num=q % NQUEUES)
                  if parts < 3:
                      return
                  for i in range(gsz):
                      w = w0 + i
                      gt4 = ggt[:, :, i, :, :]  # [128, NQ, C, elem], q strided
                      meta_t = cv.tile([128, cfg.meta_w], dt.int16, tag="meta")
                      nc.sync.dma_start(meta_t[:], ap["meta"][w, :, :])
                      dmod = meta_t[:, 0:NCH].bitcast(dt.bfloat16)
                      xblk = meta_t[:, NCH:NCH + NCH * 128].bitcast(dt.bfloat16)
                      st = cv.tile([128, NCH, 128], dt.bfloat16, tag="st")
                      st_eng = nc.gpsimd if (st2_pool and not conv1) else nc.vector
                      st_eng.tensor_tensor(
                          out=st[:],
                          in0=dmod.unsqueeze(2).to_broadcast([128, NCH, 128]),
                          in1=iota_t[:].unsqueeze(1).to_broadcast(
                              [128, NCH, 128]),
                          op=mybir.AluOpType.is_equal)
                      stT = cv.tile([128, NCH, 128], dt.bfloat16, tag="stT")
                      if stt_flat:
                          nc.vector.tensor_tensor(
                              out=stT[:].rearrange("p t s -> p (t s)"),
                              in0=xblk,
                              in1=iotapx_t[:, :NCH * 128],
                              op=mybir.AluOpType.is_equal)
                      else:
                          nc.vector.tensor_tensor(
                              out=stT[:],
                              in0=xblk.rearrange("p (t s) -> p t s", s=128),
                              in1=iotap_t[:].unsqueeze(2).to_broadcast(
                                  [128, NCH, 128]),
                              op=mybir.AluOpType.is_equal)
                      if parts < 4:
                          continue
                      ere = cvps.tile([128, NCH, nh], dt.float32, space="PSUM",
                                      tag="ere")
                      for t in range(NCH):
                          nc.tensor.matmul(out=ere[:, t, :], lhsT=stT[:, t, :],
                                           rhs=erb_t[:, w, :],
                                           start=True, stop=True)
                      el = gt4.bitcast(dt.float32)[
                          :, :, :, el_f32_off:el_f32_off + nh]
                      v = cv.tile([128, NCH, nh], dt.float32, tag="v")
                      nc.vector.tensor_tensor(
                          out=v[:].rearrange("p (q c) h -> p q c h", q=NQ),
                          in0=el,
                          in1=ere[:].rearrange("p (q c) h -> p q c h", q=NQ),
                          op=mybir.AluOpType.add)
                      e1 = cv.tile([128, NCH, nh], dt.float32, tag="e1")
                      nc.scalar.activation(e1[:], v[:],
                                           mybir.ActivationFunctionType.Exp)
                      e2 = cv.tile([128, NCH, nh], dt.float32, tag="e2")
                      nc.scalar.activation(e2[:], v[:],
                                           mybir.ActivationFunctionType.Exp,
                                           scale=NEG)
                      ee = cv.tile([128, NCH, nh], dt.bfloat16, tag="ee")
                      nc.vector.tensor_tensor(out=ee[:], in0=e1[:], in1=e2[:],
                                              op=mybir.AluOpType.max)
                      if parts < 5:
                          continue
                      msg = cv.tile([128, NCH, agg_cols], dt.bfloat16, tag="msg")
                      msg4 = msg[:].rearrange("p (q c) f -> p q c f", q=NQ)
                      ee4 = ee[:].rearrange("p (q c) h -> p q c h", q=NQ)
                      for q in range(NQ):
                          nc.vector.tensor_tensor(
                              out=msg4[:, q, :, 0:fw].rearrange(
                                  "p c (h f) -> p c h f", h=nh),
                              in0=gt4[:, q, :, feat_off:feat_off + fw].rearrange(
                                  "p c (h f) -> p c h f", h=nh),
                              in1=ee4[:, q].unsqueeze(3).to_broadcast(
                                  [128, C, nh, fw // nh]),
                              op=mybir.AluOpType.mult)
                      nc.scalar.activation(msg[:, :, fw:agg_cols], ee[:],
                                           mybir.ActivationFunctionType.Copy)
                      if parts < 6:
                          continue
                      agg = cvps.tile([128, agg_cols], dt.float32, space="PSUM",
                                      tag="agg")
                      for t in range(NCH):
                          nc.tensor.matmul(out=agg[:], lhsT=st[:, t, :],
                                           rhs=msg[:, t, :],
                                           start=(t == 0), stop=(t == NCH - 1))
                      if parts < 7:
                          continue
                      post(w, agg, cv, cvps)

              def post1(w, agg, cv, cvps):
                  dmx = cv.tile([128, HEADS], dt.float32, tag="dmx")
                  nc.vector.tensor_scalar(dmx[:], agg[:, F1:F1 + HEADS], 1e-20, None,
                                          mybir.AluOpType.max)
                  rec = cv.tile([128, HEADS], dt.float32, tag="rec")
                  nc.vector.reciprocal(rec[:], dmx[:])
                  o1 = cv.tile([128, F1], dt.float32, tag="o1")
                  nc.vector.tensor_tensor(
                      out=o1[:].rearrange("p (h f) -> p h f", h=HEADS),
                      in0=agg[:, 0:F1].rearrange("p (h f) -> p h f", h=HEADS),
                      in1=rec[:].unsqueeze(2).to_broadcast([128, HEADS, HID]),
                      op=mybir.AluOpType.mult)
                  nc.vector.tensor_tensor(out=o1[:], in0=o1[:], in1=b1_t[:],
                                          op=mybir.AluOpType.add)
                  xw = cv.tile([128, F1], dt.bfloat16, tag="xw")
                  nc.vector.tensor_scalar(xw[:], o1[:], 0.0, None,
                                          mybir.AluOpType.max)
                  if parts < 8:
                      return
                  xt = cv.tile([128, 2, 128], dt.bfloat16, tag="xt")
                  for h in range(2):
                      tp = cvps.tile([128, 128], dt.bfloat16, space="PSUM", tag="tp")
                      nc.tensor.transpose(
                          out=tp[:], in_=xw[:, h * 128:(h + 1) * 128],
                          identity=ident_t[:])
                      nc.scalar.activation(xt[:, h, :], tp[:],
                                           mybir.ActivationFunctionType.Copy)
                  f2 = cvps.tile([128, 66], dt.float32, space="PSUM", tag="f2")
                  for h in range(2):
                      nc.tensor.matmul(out=f2[:], lhsT=xt[:, h, :], rhs=w2f_t[:, h, :],
                                       start=(h == 0), stop=(h == 1))
                  stg2 = cv.tile([128, T2W], dt.bfloat16, tag="stg2")
                  if not lazy_memset or w < cv_bufs:
                      # pool recycles cv_bufs bufs; pad cols never overwritten
                      nc.gpsimd.memset(stg2[:, OUT_F + 2:T2W], 0.0)
                  nc.scalar.activation(stg2[:, 0:OUT_F], f2[:, 0:OUT_F],
                                       mybir.ActivationFunctionType.Copy)
                  nc.vector.tensor_copy(
                      stg2[:].bitcast(dt.float32)[:, OUT_F // 2:OUT_F // 2 + 1],
                      f2[:, OUT_F:OUT_F + 1])
                  nc.vector.tensor_copy(er2b_t[:, w, :], f2[:, OUT_F + 1:OUT_F + 2])
                  nc.sync.dma_start(t2_loc[w * 128:(w + 1) * 128, :], stg2[:])

              def post2(w, agg, cv, cvps):
                  dmx = cv.tile([128, 1], dt.float32, tag="dmx2")
                  nc.vector.tensor_scalar(dmx[:], agg[:, OUT_F:OUT_F + 1], 1e-20, None,
                                          mybir.AluOpType.max)
                  rec = cv.tile([128, 1], dt.float32, tag="rec2")
                  nc.vector.reciprocal(rec[:], dmx[:])
                  o2 = cv.tile([128, OUT_F], dt.float32, tag="o2")
                  nc.vector.tensor_tensor(
                      out=o2[:], in0=agg[:, 0:OUT_F],
                      in1=rec[:].to_broadcast([128, OUT_F]),
                      op=mybir.AluOpType.mult)
                  nc.vector.tensor_tensor(out=o2[:], in0=o2[:], in1=b2_t[:],
                                          op=mybir.AluOpType.add)
                  nc.vector.tensor_scalar(o2[:], o2[:], 0.0, None,
                                          mybir.AluOpType.max)
                  nc.vector.tensor_tensor(out=o2[:], in0=o2[:], in1=pw_t[:],
                                          op=mybir.AluOpType.mult)
                  nc.vector.tensor_reduce(out=out_acc[:, w:w + 1], in_=o2[:],
                                          axis=mybir.AxisListType.X,
                                          op=mybir.AluOpType.add)

              # ---------- conv1 + fc2 per window ----------
              t1_blocks = [t1[q * cfg.blkn:q * cfg.blkn + nblk_rows[q], :]
                           for q in range(NQ)]
              if stop_after >= 3:
                  with (
                      tc.tile_pool(name="cv1", bufs=cv_bufs) as cv,
                      tc.tile_pool(name="cv1ps", bufs=2, space="PSUM") as cvps,
                  ):
                      for gi, w0 in enumerate(range(0, cfg.nw, grp1)):
                          conv_group(w0, min(grp1, cfg.nw - w0), grp1,
                                     t1_blocks, er1b_t, "idx1", F1, F1 + HEADS,
                                     post1, cv, cvps, first=(gi < cv_bufs))
                  tc.strict_bb_all_engine_barrier()

              if stop_after >= 4:
                  if sim:
                      # TimelineSim can't model collectives: approximate the
                      # AllGather's local HBM write traffic with 8 DMA copies.
                      for pp_ in range(NCORES):
                          nc.sync.dma_start(
                              t2_full[pp_ * cfg.t2ln:(pp_ + 1) * cfg.t2ln, :],
                              t2_loc[:])
                  else:
                      nc.gpsimd.collective_compute(
                          "AllGather", mybir.AluOpType.bypass,
                          replica_groups=[list(range(NCORES))],
                          ins=[t2_loc[:]], outs=[t2_full[:]])
                  tc.strict_bb_all_engine_barrier()

              if stop_after >= 5:
                  t2_blocks = [t2_full[q * cfg.t2blk:(q + 1) * cfg.t2blk, :]
                               for q in range(NQ)]
                  with (
                      tc.tile_pool(name="cv2", bufs=cv_bufs) as cv,
                      tc.tile_pool(name="cv2ps", bufs=2, space="PSUM") as cvps,
                  ):
                      for gi, w0 in enumerate(range(0, cfg.nw, grp2)):
                          conv_group(w0, min(grp2, cfg.nw - w0), grp2,
                                     t2_blocks, er2b_t, "idx2", OUT_F,
                                     OUT_F + 1, post2, cv, cvps,
                                     first=(gi < cv_bufs))
                      sg = cv.tile([128, cfg.nw], dt.float32, tag="sg")
                      nc.scalar.activation(sg[:], out_acc[:],
                                           mybir.ActivationFunctionType.Sigmoid,
                                           bias=pb_t[:])
                      nc.sync.dma_start(out_core[:], sg[:])
              elif stop_after >= 3:
                  nc.sync.dma_start(out_core[:], out_acc[:])
    nc.compile()
    return nc


_CACHE = {}


def _get_nc(cfg):
    if cfg not in _CACHE:
        _CACHE[cfg] = build(cfg)
    return _CACHE[cfg]


def kernel(**inputs) -> np.ndarray:
    cfg, in_maps = prep(inputs)
    nc = _get_nc(cfg)
    res = run_bass_kernel_spmd(nc, in_maps, core_ids=list(range(NCORES)))
    n = cfg.n
    ln = cfg.ln
    out = np.empty(n, np.float32)
    for p in range(NCORES):
        oc = np.asarray(res.results[p]["out_core"])  # [128, nw]
        flat = oc.T.reshape(-1)[:ln]  # node = 128*w + part
        out[p * ln:(p + 1) * ln] = flat
    return out

